# revision 59
# baseline (speedup 1.0000x reference)
"""LRU forward kernel for Trainium2 (8 NeuronCores, batch-parallel).

Per core = one batch element:
  x_t = Lam x_{t-1} + gamma*(B u_t);  y_t = Re(C x_t) + D u_t
with Lam = r e^{i theta} diagonal. Substituting x_t = e^{i theta t} z_t
decouples the complex recurrence into two REAL scans
    z_t = r z_{t-1} + w_t,   w_t = e^{-i theta t} (gamma B u_t)
mapped onto DVE tensor_tensor_scan. Matmuls run in bf16 single-pass
(1 cyc per 128 contraction rows — fewer PE cycles than the fp8 hi/lo
3-pass DoubleRow alternative at 1.5, with comparable rounding error),
except h-tiles FPK..7 of the D*u contraction, which ride fp8 e4m3
DoubleRow matmuls (0.5 cyc/row), two per y group, buying ~13.6 us of PE
time from the error budget. The resulting total error was predicted
EXACTLY (actual device error vector + numpy-exact quantization delta;
prediction 1.9326e-2, measured 1.933e-2) — 3.4% under the 2e-2 gate,
deterministic across runs. The budget is fully spent: any further fp8
coverage (even an eighth of Cx) predicts >2e-2. Elementwise mod/demod
runs in bf16 on DVE; PSUM->bf16 conversion rides the Act engine's
copies. Values are O(1) so only the fp8 pair is scaled (dt*4, u/4,
dodging e4m3 subnormals on the small D entries).

Schedule: dependency-free warm-up matmuls lift the PE HAM clock to
2.4 GHz during the initial DMA wait; input DMAs are split across the
two HWDGE rings (Sync + Act) in consumption-deadline order, with the
first matmul's two operands desc-generating in parallel (one per ring);
chunk-0 Bu runs k-major across all 8 PSUM banks so it tracks the
arriving k-slabs; scan carries are read in-place from the previous
chunk's tile (no Act round trip); the final y group is emitted as two
256-col sub-groups to shorten the end-of-kernel Act->DMA->HBM-receipt
chain. Measured ~171 us on TRN2 (PE stream ~153 us; residual = 8 us
framework preamble + 4.5 us first-data latency + 4 us output-drain
tail + ~2 us periodic instruction-fetch bubbles). Keep elementwise
work OFF GpSimd: its tensor ops degrade concurrent DVE throughput
~2.4x (SBUF interference). The DVE chain (26.5 us/chunk) now EXCEEDS
the y-chunk period (24.2 us); it only clears because chains run ahead
during the Bu phase — measured margin on the last chunk is ~2 us, so
any further y-side shrinking must re-verify no demod-gated PE stalls
appear late in the kernel.
"""

import os

import numpy as np
import ml_dtypes
from contextlib import ExitStack

import concourse.bass as bass
import concourse.tile as tile
from concourse import mybir
from concourse.bass import ts
from concourse.bass_utils import run_bass_kernel_spmd

dt = mybir.dt
F32 = dt.float32
BF16 = dt.bfloat16
F8 = dt.float8e4
BF = ml_dtypes.bfloat16
E4 = ml_dtypes.float8_e4m3
DR = mybir.MatmulPerfMode.DoubleRow
FPK = 4             # first h k-tile of the Du contraction run in fp8
SF8 = 4.0           # fp8 operand split scale: dt*4, u/4 (dodges e4m3
                    # subnormals on the small D entries; product unscaled)

B, L, H, N = 8, 2048, 1024, 512
P = 128
LC = 512            # l-chunk
NLC = L // LC       # 4
NT = N // P         # 4 n-tiles
HT = H // P         # 8 h-tiles
NP2 = NT // 2       # 2 n-tile pairs (mod/demod packing)
NKP8 = (HT - FPK) // 2  # fp8 DoubleRow k-pairs of the Du tail
OC = H // 512       # 2 output chunks
MULT = mybir.AluOpType.mult
ADD = mybir.AluOpType.add
COPY = mybir.ActivationFunctionType.Copy

_COMPILED = {}


# --- workaround: walrus TPB_CTRL codegen rejects >1 sem wait on the Tile
# kernel-tail drain; spread overflow waits across trailing SP nops.
def _patched_drain_and_barrier(self, tick_clock, wait_clock):
    from concourse.tile import ScopedClock
    drain_inst = self.nc.sync.drain()
    wait_clock.add_sem_waits(drain_inst.ins,
                             ScopedClock({None: tick_clock.global_clock}))
    si = drain_inst.ins.sync_info
    if si is not None and si.on_wait and len(si.on_wait) > 1:
        waits = list(si.on_wait)
        si.on_wait = waits[:1]
        for w in waits[1:]:
            nop = self.nc.sync.nop()
            nsi = nop.ins.sync_info
            if nsi is None:
                nop.ins.sync_info = mybir.SyncInfo(on_wait=[w], on_update=[])
            else:
                nsi.on_wait = [w]
    self.nc.all_engine_barrier()
    assert self.sems is not None
    popped = self.nc._tile_sem_poison_stack.pop()
    assert popped is self._sem_poison
    self.nc.clear_and_free_semaphores(list(self.sems.allocated().values()))
    self.nc.all_engine_barrier()


tile.TileContext._drain_and_barrier = _patched_drain_and_barrier


_NOPCTR = [0]


def _split_waits(nc, cap=1):
    """Walrus setupSyncWait rejects instructions with more than `cap` sem
    waits; move overflow waits onto same-engine NoOps inserted before."""
    for f in nc.m.functions:
        for blk in f.blocks:
            out = []
            for inst in blk.instructions:
                si = inst.sync_info
                if si is not None and si.on_wait and len(si.on_wait) > cap:
                    waits = list(si.on_wait)
                    si.on_wait = waits[:cap]
                    rest = waits[cap:]
                    for i in range(0, len(rest), cap):
                        _NOPCTR[0] += 1
                        nop = mybir.InstNoOp(name=f"waitsplit-{_NOPCTR[0]}",
                                             ins=[], outs=[])
                        nop.engine = inst.engine
                        nop.sync_info = mybir.SyncInfo(
                            on_wait=rest[i:i + cap], on_update=[])
                        out.append(nop)
                out.append(inst)
            blk.instructions[:] = out


def _build():
    nc = bass.Bass()
    ut_ext = nc.declare_dram_parameter("ut", [H, L], BF16, isOutput=False)
    bg_ext = nc.declare_dram_parameter("bg", [2, H, N], BF16, isOutput=False)
    ct_ext = nc.declare_dram_parameter("ct", [2, N, H], BF16, isOutput=False)
    dt_ext = nc.declare_dram_parameter("dtw", [H, H], BF16, isOutput=False)
    # fp8 copies of the Du contraction tail (h-tiles FPK..7): each
    # DoubleRow matmul replaces two bf16 k-tiles per y group (the
    # remaining error budget buys ~13.6 us of PE time; exact-predicted
    # total rel err 1.933e-2 vs the 2e-2 gate)
    uf8_ext = nc.declare_dram_parameter("uf8", [2 * NKP8 * P, L], F8,
                                        isOutput=False)
    df8_ext = nc.declare_dram_parameter("df8", [2 * NKP8 * P, H], F8,
                                        isOutput=False)
    tab_ext = nc.declare_dram_parameter("tab", [2, N, L], BF16, isOutput=False)
    rcol_ext = nc.declare_dram_parameter("rcol", [N, 1], F32, isOutput=False)
    y_ext = nc.declare_dram_parameter("y", [L, H], BF16, isOutput=True)

    with tile.TileContext(nc) as tc, ExitStack() as ctx:
        wts = ctx.enter_context(tc.tile_pool(name="wts", bufs=1))
        psum = ctx.enter_context(tc.tile_pool(name="psum", bufs=2,
                                              space="PSUM"))
        psy = ctx.enter_context(tc.tile_pool(name="psy", bufs=4,
                                             space="PSUM"))
        pu = ctx.enter_context(tc.tile_pool(name="pu", bufs=4))
        pt = ctx.enter_context(tc.tile_pool(name="pt", bufs=4))
        pb = ctx.enter_context(tc.tile_pool(name="pb", bufs=4))
        pw = ctx.enter_context(tc.tile_pool(name="pw", bufs=2))
        pm = ctx.enter_context(tc.tile_pool(name="pm", bufs=2))
        px = ctx.enter_context(tc.tile_pool(name="px", bufs=2))
        py_ = ctx.enter_context(tc.tile_pool(name="py", bufs=4))
        pc = ctx.enter_context(tc.tile_pool(name="pc", bufs=1))

        # --- HAM warm-up: dependency-free matmuls on scratch SBUF run
        # during the initial DMA wait, so the real stream starts at
        # 2.4 GHz instead of paying the 3.4 us cold ramp. Results land
        # in a py-tag psum bank that the first real user clears via
        # start=True.
        wstat = wts.tile([P, P], BF16, tag="warm", name="wstat")
        wmov = wts.tile([P, 512], BF16, tag="warmm", name="wmov")
        wps = psy.tile([P, 512], F32, tag="py", name="warmps")
        nc.vector.memset(wstat[:], 0.0)
        nc.vector.memset(wmov[:], 0.0)
        NWARM = 16
        for i in range(NWARM):
            nc.tensor.matmul(wps[:], wstat[:], wmov[:],
                             start=(i == 0), stop=(i == NWARM - 1))

        # --- persistent weights, k-slab tiles ---
        # bg[c]: [P, (k n)] = [128, 8*512]; lc=0 DMA emission is
        # interleaved per k-slab with the first u chunk (fastest start).
        bg = {c: wts.tile([P, HT * N], BF16, tag=f"bg{c}", name=f"bg{c}")
              for c in range(2)}

        def bg_slab(c, k, n):
            return bg[c][:, k * N + n * P:k * N + (n + 1) * P]

        # rcol, all 4 n-tiles in one [128, 4] tile (single DMA)
        rba = pc.tile([P, NT], F32, tag="rc", name="rc")
        rb = [rba[:, n:n + 1] for n in range(NT)]

        # ct[c]: [P, (t f)] = [128, 4*1024]; dtw: [P, (k f)] = [128, 8*1024]
        # (DMAs deferred until after the last Bu block; deadline = first
        # y group.)
        ct = {c: wts.tile([P, NT * H], BF16, tag=f"ct{c}", name=f"ct{c}")
              for c in range(2)}
        dtw = wts.tile([P, HT * H], BF16, tag="dtw", name="dtw")
        df8 = wts.tile([P, NKP8 * 2 * H], F8, tag="df8", name="df8")

        def emit_wts2():
            for c in range(2):
                src = ct_ext[c, :, :].rearrange('(t p) f -> p t f', t=NT)
                nc.sync.dma_start(
                    out=ct[c][:].rearrange('p (t f) -> p t f', t=NT),
                    in_=src)
            src = dt_ext[:, :].rearrange('(k p) f -> p k f', k=HT)
            nc.sync.dma_start(
                out=dtw[:].rearrange('p (k f) -> p k f', k=HT),
                in_=src)
            nc.sync.dma_start(
                out=df8[:].rearrange('p (kp j f) -> p kp j f',
                                     kp=NKP8, j=2),
                in_=df8_ext[:, :].rearrange('(kp j p) f -> p kp j f',
                                            kp=NKP8, j=2))

        def ct_slab(c, t, oc):
            return ct[c][:, t * H + oc * 512:t * H + (oc + 1) * 512]

        def dt_slab(k, oc):
            return dtw[:, k * H + oc * 512:k * H + (oc + 1) * 512]

        # zero initial carries (chunk 0 only; later chunks read the
        # carry straight out of the previous chunk's wr/wi last column)
        c_re, c_im = [], []
        for n in range(NT):
            cr = pc.tile([P, 1], F32, tag=f"cre{n}", name=f"cre{n}")
            ci = pc.tile([P, 1], F32, tag=f"cim{n}", name=f"cim{n}")
            nc.vector.memset(cr[:], 0.0)
            nc.vector.memset(ci[:], 0.0)
            c_re.append(cr)
            c_im.append(ci)

        def emit_y(yu, yu8, yx, lci):
            """y[l, o] = x*C + u*D for chunk lci (yx = demodulated x).

            The very last (lt, oc) group of the kernel is emitted as 4
            column sub-groups so the end-of-kernel Act->DMA->HBM-receipt
            chain rides a 128-col sliver instead of the full 512.
            """
            for lt in range(LC // P):
                ys = py_.tile([P, H], BF16, tag="ystage", name="ystage")
                for oc in range(OC):
                    final = (lci == NLC - 1 and lt == LC // P - 1
                             and oc == OC - 1)
                    nsub, sw = (2, 256) if final else (1, 512)
                    for sub in range(nsub):
                        cs = oc * 512 + sub * sw
                        pyb = psy.tile([P, 512], F32, tag="py", name="psy")
                        # order: pair0 Cx, then Du, then pair1 Cx — so
                        # the group can start before the last demod.
                        # Du h-tiles FPK..7 ride fp8 DoubleRow matmuls.
                        nmm = 8 + FPK + NKP8
                        k = 0

                        def cx(np2):
                            nonlocal k
                            for c in range(2):
                                xcs = yx[c][np2]
                                for jj in range(2):
                                    t = 2 * np2 + jj
                                    stat = xcs[:, jj * LC + lt * P:
                                               jj * LC + (lt + 1) * P]
                                    k += 1
                                    nc.tensor.matmul(
                                        pyb[:, 0:sw], stat,
                                        ct[c][:, t * H + cs:t * H + cs + sw],
                                        start=(k == 1), stop=(k == nmm))

                        cx(0)
                        for kk in range(FPK):
                            stat = yu[:, kk * LC + lt * P:
                                      kk * LC + (lt + 1) * P]
                            k += 1
                            nc.tensor.matmul(
                                pyb[:, 0:sw], stat,
                                dtw[:, kk * H + cs:kk * H + cs + sw],
                                start=False, stop=(k == nmm))
                        yu8v = yu8[:].rearrange('p (kp j f) -> p kp j f',
                                                kp=NKP8, j=2)
                        df8v = df8[:].rearrange('p (kp j f) -> p kp j f',
                                                kp=NKP8, j=2)
                        for kp in range(NKP8):
                            stat8 = yu8v[:, kp, :, lt * P:(lt + 1) * P]
                            mov8 = df8v[:, kp, :, cs:cs + sw]
                            k += 1
                            nc.tensor.matmul(pyb[:, 0:sw], stat8, mov8,
                                             start=False,
                                             stop=(k == nmm),
                                             perf_mode=DR)
                        cx(1)
                        nc.scalar.activation(ys[:, cs:cs + sw],
                                             pyb[:, 0:sw], COPY)
                        nc.sync.dma_start(
                            out=y_ext[ts(lci * (LC // P) + lt, P),
                                      cs:cs + sw],
                            in_=ys[:, cs:cs + sw])

        # ---- phase 1: DMAs + Bu matmuls + Act psum->bf16 copies ----
        # All input DMAs are emitted up front, split across the two
        # HWDGE rings (Sync + Act) and ordered by consumption deadline:
        #   Sync:   bg0/u0 k-slabs interleaved, tab1, u3, tab2, tab3,
        #           ct/dt, then y outputs
        #   Scalar: bg1 k-slabs, rcol, u1, tab0, u2, then Act copies
        # so no consumer ever queues behind desc-gen for later traffic.
        # All four Bu phases then run back-to-back on the PE. Phase 2
        # runs chain(k) + y(k) per chunk.
        ucs = [pu.tile([P, HT * LC], BF16, tag="u", name=f"u{lc}")
               for lc in range(NLC)]
        uf8s = [pu.tile([P, NKP8 * 2 * LC], F8, tag="uf8",
                        name=f"uf8_{lc}")
                for lc in range(NLC)]
        tabs = [[pt.tile([P, NP2 * 2 * LC], BF16, tag=f"tab{tt}",
                         name=f"tab{tt}_{lc}") for tt in range(2)]
                for lc in range(NLC)]

        def emit_u_dma(eng, lc):
            if lc == 0:
                return
            src = ut_ext[:, ts(lc, LC)].rearrange('(k p) f -> p k f', k=HT)
            eng.dma_start(
                out=ucs[lc][:].rearrange('p (k f) -> p k f', k=HT),
                in_=src)

        def emit_tab_dma(eng, lc):
            for tt in range(2):
                src = tab_ext[tt, :, ts(lc, LC)].rearrange(
                    '(q j p) f -> p q j f', q=NP2, j=2)
                eng.dma_start(
                    out=tabs[lc][tt][:].rearrange('p (q j f) -> p q j f',
                                                  q=NP2, j=2),
                    in_=src)

        # singles for k=0,1 (earliest possible first matmul), pairs
        # after (fewer desc-gens + less DMA-sem lane recycling). u0k0
        # rides the Scalar ring so the first matmul's two operands
        # desc-gen in PARALLEL instead of serializing on Sync.
        slabs = [(0, 1), (1, 1), (2, 2), (4, 2), (6, 2)]

        def slab_dma(eng, ext, tile_, k0, kw, w):
            src = ext[k0 * P:(k0 + kw) * P, :].rearrange(
                '(k p) f -> p k f', k=kw)
            eng.dma_start(
                out=tile_[:, k0 * w:(k0 + kw) * w].rearrange(
                    'p (k f) -> p k f', k=kw),
                in_=src)

        u0view = ut_ext[:, 0:LC]
        slab_dma(nc.scalar, u0view, ucs[0], 0, 1, LC)
        slab_dma(nc.sync, bg_ext[0], bg[0], 0, 1, N)
        for k0, kw in slabs[1:]:
            slab_dma(nc.sync, bg_ext[0], bg[0], k0, kw, N)
            slab_dma(nc.sync, u0view, ucs[0], k0, kw, LC)
        for k0, kw in slabs:
            slab_dma(nc.scalar, bg_ext[1], bg[1], k0, kw, N)
        nc.scalar.dma_start(
            out=rba[:],
            in_=rcol_ext[:, :].rearrange('(t p) f -> p (t f)', t=NT))
        emit_u_dma(nc.scalar, 1)
        emit_tab_dma(nc.scalar, 0)
        emit_u_dma(nc.scalar, 2)
        emit_tab_dma(nc.sync, 1)
        emit_u_dma(nc.sync, 3)
        emit_tab_dma(nc.sync, 2)
        emit_tab_dma(nc.sync, 3)
        emit_wts2()
        for lc in range(NLC):
            nc.sync.dma_start(
                out=uf8s[lc][:].rearrange('p (kp j f) -> p kp j f',
                                          kp=NKP8, j=2),
                in_=uf8_ext[:, ts(lc, LC)].rearrange(
                    '(kp j p) f -> p kp j f', kp=NKP8, j=2))

        stages = []
        for lc in range(NLC):
            uc = ucs[lc]
            tabt = tabs[lc]
            # PE: Bu matmuls per n-tile (out partitions = n).
            bups = {}
            if lc == 0:
                # k-major with all 8 psum groups open (4 borrowed from
                # the idle y-psum banks): each DMA arrival unblocks a
                # whole k-slab pass instead of serializing behind
                # per-group late passes — Bu0 tracks the DMA stream.
                order = ([(n, 0) for n in range(NT)]
                         + [(n, 1) for n in range(NT)])
                for i, (n, c) in enumerate(order):
                    # 8 distinct PSUM banks: pbu0/pbu1 have 2 bufs each
                    # (rotation gives n=0..3 distinct banks), py has 4.
                    pool_, tag_ = ((psum, f"pbu{n % 2}") if i < 4 else
                                   (psy, "py"))
                    bups[n, c] = pool_.tile([P, LC], F32, tag=tag_,
                                            name=f"bu0_{i}")
                for k in range(HT):
                    for n, c in order:
                        nc.tensor.matmul(
                            bups[n, c][:], bg_slab(c, k, n),
                            uc[:, ts(k, LC)],
                            start=(k == 0), stop=(k == HT - 1))
            else:
                for n in range(NT):
                    for c in range(2):
                        ps = psum.tile([P, LC], F32, tag=f"pbu{c}",
                                       name=f"pbu{c}")
                        for k in range(HT):
                            nc.tensor.matmul(ps[:], bg_slab(c, k, n),
                                             uc[:, ts(k, LC)],
                                             start=(k == 0),
                                             stop=(k == HT - 1))
                        bups[n, c] = ps
            burs = []
            for np2 in range(NP2):
                bur = pb.tile([P, 2 * LC], BF16, tag=f"bur{np2}",
                              name=f"bur{np2}")
                bui = pb.tile([P, 2 * LC], BF16, tag=f"bui{np2}",
                              name=f"bui{np2}")
                for jj in range(2):
                    n = 2 * np2 + jj
                    nc.scalar.activation(bur[:, ts(jj, LC)], bups[n, 0][:],
                                         COPY)
                    nc.scalar.activation(bui[:, ts(jj, LC)], bups[n, 1][:],
                                         COPY)
                burs.append((bur, bui))
            stages.append((uc, tabt, burs))

        # ---- phase 2: vector chain(k) then y(k), per chunk ----
        prev_w = {}  # np2 -> (wr, wi) of previous chunk (carry source)
        for lc in range(NLC):
            uc, tabt, burs = stages[lc]
            xs = {0: [], 1: []}  # [c] -> per-np2 bf16 tiles [P, 2*LC]
            for np2 in range(NP2):
                cosr = tabt[0][:, ts(np2, 2 * LC)]
                sinr = tabt[1][:, ts(np2, 2 * LC)]
                n0, n1 = 2 * np2, 2 * np2 + 1
                bur, bui = burs[np2]
                # mod: w = conj(E) * Bu. (All elementwise work stays on
                # DVE: GpSimd tensor ops degrade DVE throughput ~2.4x
                # via SBUF port interference — measured, do not offload.)
                t1 = pm.tile([P, 2 * LC], BF16, tag="t1", name="t1")
                t2 = pm.tile([P, 2 * LC], BF16, tag="t2", name="t2")
                wr = pw.tile([P, 2 * LC], BF16, tag=f"wr{np2}",
                             name=f"wr{np2}")
                wi = pw.tile([P, 2 * LC], BF16, tag=f"wi{np2}",
                             name=f"wi{np2}")
                nc.vector.tensor_mul(t1[:], cosr, bur[:])
                nc.vector.tensor_mul(t2[:], sinr, bui[:])
                nc.vector.tensor_add(wr[:], t1[:], t2[:])
                t3 = pm.tile([P, 2 * LC], BF16, tag="t1", name="t1")
                t4 = pm.tile([P, 2 * LC], BF16, tag="t2", name="t2")
                nc.vector.tensor_mul(t3[:], cosr, bui[:])
                nc.vector.tensor_mul(t4[:], sinr, bur[:])
                nc.vector.tensor_sub(wi[:], t3[:], t4[:])
                # chunk scans with carry (state fp32 internal). The
                # carry-in is the last column of the previous chunk's
                # wr/wi tile — no copy, no Act round trip.
                for jj, n in ((0, n0), (1, n1)):
                    rbc = rb[n][:, 0:1].broadcast_to([P, LC])
                    if lc == 0:
                        ir = c_re[n][:, 0:1]
                        ii = c_im[n][:, 0:1]
                    else:
                        pwr, pwi = prev_w[np2]
                        ir = pwr[:, (jj + 1) * LC - 1:(jj + 1) * LC]
                        ii = pwi[:, (jj + 1) * LC - 1:(jj + 1) * LC]
                    nc.vector.tensor_tensor_scan(
                        wr[:, ts(jj, LC)], rbc, wr[:, ts(jj, LC)],
                        ir, MULT, ADD)
                    nc.vector.tensor_tensor_scan(
                        wi[:, ts(jj, LC)], rbc, wi[:, ts(jj, LC)],
                        ii, MULT, ADD)
                # demod: x = E * z, straight into matmul-ready bf16 tiles
                t5 = pm.tile([P, 2 * LC], BF16, tag="t1", name="t1")
                t6 = pm.tile([P, 2 * LC], BF16, tag="t2", name="t2")
                xrb = px.tile([P, 2 * LC], BF16, tag=f"xr{np2}",
                              name=f"xr{np2}")
                xib = px.tile([P, 2 * LC], BF16, tag=f"xi{np2}",
                              name=f"xi{np2}")
                nc.vector.tensor_mul(t5[:], cosr, wr[:])
                nc.vector.tensor_mul(t6[:], sinr, wi[:])
                nc.vector.tensor_sub(xrb[:], t5[:], t6[:])
                t7 = pm.tile([P, 2 * LC], BF16, tag="t1", name="t1")
                t8 = pm.tile([P, 2 * LC], BF16, tag="t2", name="t2")
                nc.vector.tensor_mul(t7[:], cosr, wi[:])
                nc.vector.tensor_mul(t8[:], sinr, wr[:])
                nc.vector.tensor_add(xib[:], t7[:], t8[:])
                xs[0].append(xrb)
                xs[1].append(xib)
                prev_w[np2] = (wr, wi)
            emit_y(uc, uf8s[lc], xs, lc)
    _split_waits(nc)
    return nc


def _prep(u, nu_log, theta_log, gamma_log, B_re, B_im, C_re, C_im, D):
    r = np.exp(-np.exp(nu_log.astype(np.float64)))
    theta = np.exp(theta_log.astype(np.float64))
    gamma = np.exp(gamma_log.astype(np.float64))
    t = np.arange(L, dtype=np.float64)
    ang = theta[:, None] * t[None, :]
    tab = np.empty((2, N, L), BF)
    tab[0] = np.cos(ang).astype(BF)
    tab[1] = np.sin(ang).astype(BF)
    rcol = r.astype(np.float32)[:, None].copy()
    bgt = np.empty((2, H, N), BF)
    bgt[0] = (gamma[:, None] * B_re).T.astype(BF)
    bgt[1] = (gamma[:, None] * B_im).T.astype(BF)
    ctt = np.empty((2, N, H), BF)
    ctt[0] = C_re.T.astype(BF)
    ctt[1] = (-C_im.T).astype(BF)
    dtn = np.ascontiguousarray(D.T)
    dtw = dtn.astype(BF)
    df8 = (dtn[FPK * 128:, :] * SF8).astype(E4)
    common = dict(bg=bgt, ct=ctt, dtw=dtw, df8=df8, tab=tab, rcol=rcol)
    in_maps = []
    for b in range(B):
        m = dict(common)
        ut = np.ascontiguousarray(u[b].T)
        m["ut"] = ut.astype(BF)
        m["uf8"] = (ut[FPK * 128:, :] / SF8).astype(E4)
        in_maps.append(m)
    return in_maps


def kernel(u, nu_log, theta_log, gamma_log, B_re, B_im, C_re, C_im, D,
           _trace=False):
    u, nu_log, theta_log, gamma_log, B_re, B_im, C_re, C_im, D = (
        np.asarray(a) for a in
        (u, nu_log, theta_log, gamma_log, B_re, B_im, C_re, C_im, D))
    if "nc" not in _COMPILED:
        _COMPILED["nc"] = _build()
    nc = _COMPILED["nc"]
    in_maps = _prep(u, nu_log, theta_log, gamma_log, B_re, B_im, C_re, C_im, D)
    res = None
    err = None
    for _attempt in range(4):
        try:
            res = run_bass_kernel_spmd(nc, in_maps, list(range(B)),
                                       trace=_trace)
            break
        except ModuleNotFoundError:
            # axon NTFF hook unavailable; force tracing off (BASS_TRACE
            # in the env would otherwise re-enable it) and rerun
            _trace = False
            os.environ["BASS_NEVER_TRACE"] = "1"
        except Exception as e:  # transient NRT / device hiccup: retry
            err = e
    if res is None:
        raise err if err is not None else RuntimeError("bass run failed")
    y = np.stack([res.results[i]["y"].astype(np.float32) for i in range(B)])
    kernel.last_exec_time_ns = res.exec_time_ns
    return y


# revision 60
# speedup vs baseline: 1.0031x; 1.0031x over previous
"""LRU forward kernel for Trainium2 (8 NeuronCores, batch-parallel).

Per core = one batch element:
  x_t = Lam x_{t-1} + gamma*(B u_t);  y_t = Re(C x_t) + D u_t
with Lam = r e^{i theta} diagonal. Substituting x_t = e^{i theta t} z_t
decouples the complex recurrence into two REAL scans
    z_t = r z_{t-1} + w_t,   w_t = e^{-i theta t} (gamma B u_t)
mapped onto DVE tensor_tensor_scan. Matmuls run in bf16 single-pass
(1 cyc per 128 contraction rows — fewer PE cycles than the fp8 hi/lo
3-pass DoubleRow alternative at 1.5, with comparable rounding error),
except h-tiles FPK..7 of the D*u contraction, which ride fp8 e4m3
DoubleRow matmuls (0.5 cyc/row), two per y group, buying ~13.6 us of PE
time from the error budget. The resulting total error was predicted
EXACTLY (actual device error vector + numpy-exact quantization delta;
prediction 1.9326e-2, measured 1.933e-2) — 3.4% under the 2e-2 gate,
deterministic across runs. The budget is fully spent: any further fp8
coverage (even an eighth of Cx) predicts >2e-2. Elementwise mod/demod
runs in bf16 on DVE; PSUM->bf16 conversion rides the Act engine's
copies. Values are O(1) so only the fp8 pair is scaled (dt*4, u/4,
dodging e4m3 subnormals on the small D entries).

Schedule: dependency-free warm-up matmuls lift the PE HAM clock to
2.4 GHz during the initial DMA wait; input DMAs are split across the
two HWDGE rings (Sync + Act) in consumption-deadline order, with the
first matmul's two operands desc-generating in parallel (one per ring);
chunk-0 Bu runs k-major across all 8 PSUM banks so it tracks the
arriving k-slabs; scan carries are read in-place from the previous
chunk's tile (no Act round trip); the final y group is emitted as two
256-col sub-groups to shorten the end-of-kernel Act->DMA->HBM-receipt
chain. Measured ~171 us on TRN2 (PE stream ~153 us; residual = 8 us
framework preamble + 4.5 us first-data latency + 4 us output-drain
tail + ~2 us periodic instruction-fetch bubbles). Keep elementwise
work OFF GpSimd: its tensor ops degrade concurrent DVE throughput
~2.4x (SBUF interference). The DVE chain (26.5 us/chunk) now EXCEEDS
the y-chunk period (24.2 us); it only clears because chains run ahead
during the Bu phase — measured margin on the last chunk is ~2 us, so
any further y-side shrinking must re-verify no demod-gated PE stalls
appear late in the kernel.
"""

import os

import numpy as np
import ml_dtypes
from contextlib import ExitStack

import concourse.bass as bass
import concourse.tile as tile
from concourse import mybir
from concourse.bass import ts
from concourse.bass_utils import run_bass_kernel_spmd

dt = mybir.dt
F32 = dt.float32
BF16 = dt.bfloat16
F8 = dt.float8e4
BF = ml_dtypes.bfloat16
E4 = ml_dtypes.float8_e4m3
DR = mybir.MatmulPerfMode.DoubleRow
FPK = 4             # first h k-tile of the Du contraction run in fp8
SF8 = 4.0           # fp8 operand split scale: dt*4, u/4 (dodges e4m3
                    # subnormals on the small D entries; product unscaled)

B, L, H, N = 8, 2048, 1024, 512
P = 128
LC = 512            # l-chunk
NLC = L // LC       # 4
NT = N // P         # 4 n-tiles
HT = H // P         # 8 h-tiles
NP2 = NT // 2       # 2 n-tile pairs (mod/demod packing)
NKP8 = (HT - FPK) // 2  # fp8 DoubleRow k-pairs of the Du tail
OC = H // 512       # 2 output chunks
MULT = mybir.AluOpType.mult
ADD = mybir.AluOpType.add
COPY = mybir.ActivationFunctionType.Copy

_COMPILED = {}


# --- workaround: walrus TPB_CTRL codegen rejects >1 sem wait on the Tile
# kernel-tail drain; spread overflow waits across trailing SP nops.
def _patched_drain_and_barrier(self, tick_clock, wait_clock):
    from concourse.tile import ScopedClock
    drain_inst = self.nc.sync.drain()
    wait_clock.add_sem_waits(drain_inst.ins,
                             ScopedClock({None: tick_clock.global_clock}))
    si = drain_inst.ins.sync_info
    if si is not None and si.on_wait and len(si.on_wait) > 1:
        waits = list(si.on_wait)
        si.on_wait = waits[:1]
        for w in waits[1:]:
            nop = self.nc.sync.nop()
            nsi = nop.ins.sync_info
            if nsi is None:
                nop.ins.sync_info = mybir.SyncInfo(on_wait=[w], on_update=[])
            else:
                nsi.on_wait = [w]
    self.nc.all_engine_barrier()
    assert self.sems is not None
    popped = self.nc._tile_sem_poison_stack.pop()
    assert popped is self._sem_poison
    self.nc.clear_and_free_semaphores(list(self.sems.allocated().values()))
    self.nc.all_engine_barrier()


tile.TileContext._drain_and_barrier = _patched_drain_and_barrier


_NOPCTR = [0]


def _split_waits(nc, cap=1):
    """Walrus setupSyncWait rejects instructions with more than `cap` sem
    waits; move overflow waits onto same-engine NoOps inserted before."""
    for f in nc.m.functions:
        for blk in f.blocks:
            out = []
            for inst in blk.instructions:
                si = inst.sync_info
                if si is not None and si.on_wait and len(si.on_wait) > cap:
                    waits = list(si.on_wait)
                    si.on_wait = waits[:cap]
                    rest = waits[cap:]
                    for i in range(0, len(rest), cap):
                        _NOPCTR[0] += 1
                        nop = mybir.InstNoOp(name=f"waitsplit-{_NOPCTR[0]}",
                                             ins=[], outs=[])
                        nop.engine = inst.engine
                        nop.sync_info = mybir.SyncInfo(
                            on_wait=rest[i:i + cap], on_update=[])
                        out.append(nop)
                out.append(inst)
            blk.instructions[:] = out


def _build():
    nc = bass.Bass()
    ut_ext = nc.declare_dram_parameter("ut", [H, L], BF16, isOutput=False)
    bg_ext = nc.declare_dram_parameter("bg", [2, H, N], BF16, isOutput=False)
    ct_ext = nc.declare_dram_parameter("ct", [2, N, H], BF16, isOutput=False)
    dt_ext = nc.declare_dram_parameter("dtw", [H, H], BF16, isOutput=False)
    # fp8 copies of the Du contraction tail (h-tiles FPK..7): each
    # DoubleRow matmul replaces two bf16 k-tiles per y group (the
    # remaining error budget buys ~13.6 us of PE time; exact-predicted
    # total rel err 1.933e-2 vs the 2e-2 gate)
    uf8_ext = nc.declare_dram_parameter("uf8", [2 * NKP8 * P, L], F8,
                                        isOutput=False)
    df8_ext = nc.declare_dram_parameter("df8", [2 * NKP8 * P, H], F8,
                                        isOutput=False)
    tab_ext = nc.declare_dram_parameter("tab", [2, N, L], BF16, isOutput=False)
    rcol_ext = nc.declare_dram_parameter("rcol", [N, 1], F32, isOutput=False)
    y_ext = nc.declare_dram_parameter("y", [L, H], BF16, isOutput=True)

    with tile.TileContext(nc) as tc, ExitStack() as ctx:
        wts = ctx.enter_context(tc.tile_pool(name="wts", bufs=1))
        psum = ctx.enter_context(tc.tile_pool(name="psum", bufs=2,
                                              space="PSUM"))
        psy = ctx.enter_context(tc.tile_pool(name="psy", bufs=4,
                                             space="PSUM"))
        pu = ctx.enter_context(tc.tile_pool(name="pu", bufs=4))
        pt = ctx.enter_context(tc.tile_pool(name="pt", bufs=4))
        pb = ctx.enter_context(tc.tile_pool(name="pb", bufs=4))
        pw = ctx.enter_context(tc.tile_pool(name="pw", bufs=2))
        pm = ctx.enter_context(tc.tile_pool(name="pm", bufs=2))
        px = ctx.enter_context(tc.tile_pool(name="px", bufs=2))
        py_ = ctx.enter_context(tc.tile_pool(name="py", bufs=4))
        pc = ctx.enter_context(tc.tile_pool(name="pc", bufs=1))

        # --- HAM warm-up: dependency-free matmuls on scratch SBUF run
        # during the initial DMA wait, so the real stream starts at
        # 2.4 GHz instead of paying the 3.4 us cold ramp. Results land
        # in a py-tag psum bank that the first real user clears via
        # start=True.
        wstat = wts.tile([P, P], BF16, tag="warm", name="wstat")
        wmov = wts.tile([P, 512], BF16, tag="warmm", name="wmov")
        wps = psy.tile([P, 512], F32, tag="py", name="warmps")
        nc.gpsimd.memset(wstat[:], 0.0)
        nc.gpsimd.memset(wmov[:], 0.0)
        NWARM = 13
        for i in range(NWARM):
            nc.tensor.matmul(wps[:], wstat[:], wmov[:],
                             start=(i == 0), stop=(i == NWARM - 1))

        # --- persistent weights, k-slab tiles ---
        # bg[c]: [P, (k n)] = [128, 8*512]; lc=0 DMA emission is
        # interleaved per k-slab with the first u chunk (fastest start).
        bg = {c: wts.tile([P, HT * N], BF16, tag=f"bg{c}", name=f"bg{c}")
              for c in range(2)}

        def bg_slab(c, k, n):
            return bg[c][:, k * N + n * P:k * N + (n + 1) * P]

        # rcol, all 4 n-tiles in one [128, 4] tile (single DMA)
        rba = pc.tile([P, NT], F32, tag="rc", name="rc")
        rb = [rba[:, n:n + 1] for n in range(NT)]

        # ct[c]: [P, (t f)] = [128, 4*1024]; dtw: [P, (k f)] = [128, 8*1024]
        # (DMAs deferred until after the last Bu block; deadline = first
        # y group.)
        ct = {c: wts.tile([P, NT * H], BF16, tag=f"ct{c}", name=f"ct{c}")
              for c in range(2)}
        dtw = wts.tile([P, HT * H], BF16, tag="dtw", name="dtw")
        df8 = wts.tile([P, NKP8 * 2 * H], F8, tag="df8", name="df8")

        def emit_wts2():
            for c in range(2):
                src = ct_ext[c, :, :].rearrange('(t p) f -> p t f', t=NT)
                nc.sync.dma_start(
                    out=ct[c][:].rearrange('p (t f) -> p t f', t=NT),
                    in_=src)
            src = dt_ext[:, :].rearrange('(k p) f -> p k f', k=HT)
            nc.sync.dma_start(
                out=dtw[:].rearrange('p (k f) -> p k f', k=HT),
                in_=src)
            nc.sync.dma_start(
                out=df8[:].rearrange('p (kp j f) -> p kp j f',
                                     kp=NKP8, j=2),
                in_=df8_ext[:, :].rearrange('(kp j p) f -> p kp j f',
                                            kp=NKP8, j=2))

        def ct_slab(c, t, oc):
            return ct[c][:, t * H + oc * 512:t * H + (oc + 1) * 512]

        def dt_slab(k, oc):
            return dtw[:, k * H + oc * 512:k * H + (oc + 1) * 512]

        # zero initial carries (chunk 0 only; later chunks read the
        # carry straight out of the previous chunk's wr/wi last column)
        c_re, c_im = [], []
        for n in range(NT):
            cr = pc.tile([P, 1], F32, tag=f"cre{n}", name=f"cre{n}")
            ci = pc.tile([P, 1], F32, tag=f"cim{n}", name=f"cim{n}")
            nc.vector.memset(cr[:], 0.0)
            nc.vector.memset(ci[:], 0.0)
            c_re.append(cr)
            c_im.append(ci)

        def emit_y(yu, yu8, yx, lci):
            """y[l, o] = x*C + u*D for chunk lci (yx = demodulated x).

            The very last (lt, oc) group of the kernel is emitted as 4
            column sub-groups so the end-of-kernel Act->DMA->HBM-receipt
            chain rides a 128-col sliver instead of the full 512.
            """
            for lt in range(LC // P):
                ys = py_.tile([P, H], BF16, tag="ystage", name="ystage")
                for oc in range(OC):
                    final = (lci == NLC - 1 and lt == LC // P - 1
                             and oc == OC - 1)
                    nsub, sw = (2, 256) if final else (1, 512)
                    for sub in range(nsub):
                        cs = oc * 512 + sub * sw
                        pyb = psy.tile([P, 512], F32, tag="py", name="psy")
                        # order: pair0 Cx, then Du, then pair1 Cx — so
                        # the group can start before the last demod.
                        # Du h-tiles FPK..7 ride fp8 DoubleRow matmuls.
                        nmm = 8 + FPK + NKP8
                        k = 0

                        def cx(np2):
                            nonlocal k
                            for c in range(2):
                                xcs = yx[c][np2]
                                for jj in range(2):
                                    t = 2 * np2 + jj
                                    stat = xcs[:, jj * LC + lt * P:
                                               jj * LC + (lt + 1) * P]
                                    k += 1
                                    nc.tensor.matmul(
                                        pyb[:, 0:sw], stat,
                                        ct[c][:, t * H + cs:t * H + cs + sw],
                                        start=(k == 1), stop=(k == nmm))

                        cx(0)
                        for kk in range(FPK):
                            stat = yu[:, kk * LC + lt * P:
                                      kk * LC + (lt + 1) * P]
                            k += 1
                            nc.tensor.matmul(
                                pyb[:, 0:sw], stat,
                                dtw[:, kk * H + cs:kk * H + cs + sw],
                                start=False, stop=(k == nmm))
                        yu8v = yu8[:].rearrange('p (kp j f) -> p kp j f',
                                                kp=NKP8, j=2)
                        df8v = df8[:].rearrange('p (kp j f) -> p kp j f',
                                                kp=NKP8, j=2)
                        for kp in range(NKP8):
                            stat8 = yu8v[:, kp, :, lt * P:(lt + 1) * P]
                            mov8 = df8v[:, kp, :, cs:cs + sw]
                            k += 1
                            nc.tensor.matmul(pyb[:, 0:sw], stat8, mov8,
                                             start=False,
                                             stop=(k == nmm),
                                             perf_mode=DR)
                        cx(1)
                        nc.scalar.activation(ys[:, cs:cs + sw],
                                             pyb[:, 0:sw], COPY)
                        nc.sync.dma_start(
                            out=y_ext[ts(lci * (LC // P) + lt, P),
                                      cs:cs + sw],
                            in_=ys[:, cs:cs + sw])

        # ---- phase 1: DMAs + Bu matmuls + Act psum->bf16 copies ----
        # All input DMAs are emitted up front, split across the two
        # HWDGE rings (Sync + Act) and ordered by consumption deadline:
        #   Sync:   bg0/u0 k-slabs interleaved, tab1, u3, tab2, tab3,
        #           ct/dt, then y outputs
        #   Scalar: bg1 k-slabs, rcol, u1, tab0, u2, then Act copies
        # so no consumer ever queues behind desc-gen for later traffic.
        # All four Bu phases then run back-to-back on the PE. Phase 2
        # runs chain(k) + y(k) per chunk.
        ucs = [pu.tile([P, HT * LC], BF16, tag="u", name=f"u{lc}")
               for lc in range(NLC)]
        uf8s = [pu.tile([P, NKP8 * 2 * LC], F8, tag="uf8",
                        name=f"uf8_{lc}")
                for lc in range(NLC)]
        tabs = [[pt.tile([P, NP2 * 2 * LC], BF16, tag=f"tab{tt}",
                         name=f"tab{tt}_{lc}") for tt in range(2)]
                for lc in range(NLC)]

        def emit_u_dma(eng, lc):
            if lc == 0:
                return
            src = ut_ext[:, ts(lc, LC)].rearrange('(k p) f -> p k f', k=HT)
            eng.dma_start(
                out=ucs[lc][:].rearrange('p (k f) -> p k f', k=HT),
                in_=src)

        def emit_tab_dma(eng, lc):
            for tt in range(2):
                src = tab_ext[tt, :, ts(lc, LC)].rearrange(
                    '(q j p) f -> p q j f', q=NP2, j=2)
                eng.dma_start(
                    out=tabs[lc][tt][:].rearrange('p (q j f) -> p q j f',
                                                  q=NP2, j=2),
                    in_=src)

        # singles for k=0,1 (earliest possible first matmul), pairs
        # after (fewer desc-gens + less DMA-sem lane recycling). u0k0
        # rides the Scalar ring so the first matmul's two operands
        # desc-gen in PARALLEL instead of serializing on Sync.
        slabs = [(0, 1), (1, 1), (2, 2), (4, 2), (6, 2)]

        def slab_dma(eng, ext, tile_, k0, kw, w):
            src = ext[k0 * P:(k0 + kw) * P, :].rearrange(
                '(k p) f -> p k f', k=kw)
            eng.dma_start(
                out=tile_[:, k0 * w:(k0 + kw) * w].rearrange(
                    'p (k f) -> p k f', k=kw),
                in_=src)

        u0view = ut_ext[:, 0:LC]
        slab_dma(nc.scalar, u0view, ucs[0], 0, 1, LC)
        slab_dma(nc.sync, bg_ext[0], bg[0], 0, 1, N)
        for k0, kw in slabs[1:]:
            slab_dma(nc.sync, bg_ext[0], bg[0], k0, kw, N)
            slab_dma(nc.sync, u0view, ucs[0], k0, kw, LC)
        for k0, kw in slabs:
            slab_dma(nc.scalar, bg_ext[1], bg[1], k0, kw, N)
        nc.scalar.dma_start(
            out=rba[:],
            in_=rcol_ext[:, :].rearrange('(t p) f -> p (t f)', t=NT))
        emit_u_dma(nc.scalar, 1)
        emit_tab_dma(nc.scalar, 0)
        emit_u_dma(nc.scalar, 2)
        emit_tab_dma(nc.sync, 1)
        emit_u_dma(nc.sync, 3)
        emit_tab_dma(nc.sync, 2)
        emit_tab_dma(nc.sync, 3)
        emit_wts2()
        for lc in range(NLC):
            nc.sync.dma_start(
                out=uf8s[lc][:].rearrange('p (kp j f) -> p kp j f',
                                          kp=NKP8, j=2),
                in_=uf8_ext[:, ts(lc, LC)].rearrange(
                    '(kp j p) f -> p kp j f', kp=NKP8, j=2))

        stages = []
        for lc in range(NLC):
            uc = ucs[lc]
            tabt = tabs[lc]
            # PE: Bu matmuls per n-tile (out partitions = n).
            bups = {}
            if lc == 0:
                # k-major with all 8 psum groups open (4 borrowed from
                # the idle y-psum banks): each DMA arrival unblocks a
                # whole k-slab pass instead of serializing behind
                # per-group late passes — Bu0 tracks the DMA stream.
                order = ([(n, 0) for n in range(NT)]
                         + [(n, 1) for n in range(NT)])
                for i, (n, c) in enumerate(order):
                    # 8 distinct PSUM banks: pbu0/pbu1 have 2 bufs each
                    # (rotation gives n=0..3 distinct banks), py has 4.
                    pool_, tag_ = ((psum, f"pbu{n % 2}") if i < 4 else
                                   (psy, "py"))
                    bups[n, c] = pool_.tile([P, LC], F32, tag=tag_,
                                            name=f"bu0_{i}")
                for k in range(HT):
                    for n, c in order:
                        nc.tensor.matmul(
                            bups[n, c][:], bg_slab(c, k, n),
                            uc[:, ts(k, LC)],
                            start=(k == 0), stop=(k == HT - 1))
            else:
                for n in range(NT):
                    for c in range(2):
                        ps = psum.tile([P, LC], F32, tag=f"pbu{c}",
                                       name=f"pbu{c}")
                        for k in range(HT):
                            nc.tensor.matmul(ps[:], bg_slab(c, k, n),
                                             uc[:, ts(k, LC)],
                                             start=(k == 0),
                                             stop=(k == HT - 1))
                        bups[n, c] = ps
            burs = []
            for np2 in range(NP2):
                bur = pb.tile([P, 2 * LC], BF16, tag=f"bur{np2}",
                              name=f"bur{np2}")
                bui = pb.tile([P, 2 * LC], BF16, tag=f"bui{np2}",
                              name=f"bui{np2}")
                for jj in range(2):
                    n = 2 * np2 + jj
                    nc.scalar.activation(bur[:, ts(jj, LC)], bups[n, 0][:],
                                         COPY)
                    nc.scalar.activation(bui[:, ts(jj, LC)], bups[n, 1][:],
                                         COPY)
                burs.append((bur, bui))
            stages.append((uc, tabt, burs))

        # ---- phase 2: vector chain(k) then y(k), per chunk ----
        prev_w = {}  # np2 -> (wr, wi) of previous chunk (carry source)
        for lc in range(NLC):
            uc, tabt, burs = stages[lc]
            xs = {0: [], 1: []}  # [c] -> per-np2 bf16 tiles [P, 2*LC]
            for np2 in range(NP2):
                cosr = tabt[0][:, ts(np2, 2 * LC)]
                sinr = tabt[1][:, ts(np2, 2 * LC)]
                n0, n1 = 2 * np2, 2 * np2 + 1
                bur, bui = burs[np2]
                # mod: w = conj(E) * Bu. (All elementwise work stays on
                # DVE: GpSimd tensor ops degrade DVE throughput ~2.4x
                # via SBUF port interference — measured, do not offload.)
                t1 = pm.tile([P, 2 * LC], BF16, tag="t1", name="t1")
                t2 = pm.tile([P, 2 * LC], BF16, tag="t2", name="t2")
                wr = pw.tile([P, 2 * LC], BF16, tag=f"wr{np2}",
                             name=f"wr{np2}")
                wi = pw.tile([P, 2 * LC], BF16, tag=f"wi{np2}",
                             name=f"wi{np2}")
                nc.vector.tensor_mul(t1[:], cosr, bur[:])
                nc.vector.tensor_mul(t2[:], sinr, bui[:])
                nc.vector.tensor_add(wr[:], t1[:], t2[:])
                t3 = pm.tile([P, 2 * LC], BF16, tag="t1", name="t1")
                t4 = pm.tile([P, 2 * LC], BF16, tag="t2", name="t2")
                nc.vector.tensor_mul(t3[:], cosr, bui[:])
                nc.vector.tensor_mul(t4[:], sinr, bur[:])
                nc.vector.tensor_sub(wi[:], t3[:], t4[:])
                # chunk scans with carry (state fp32 internal). The
                # carry-in is the last column of the previous chunk's
                # wr/wi tile — no copy, no Act round trip.
                for jj, n in ((0, n0), (1, n1)):
                    rbc = rb[n][:, 0:1].broadcast_to([P, LC])
                    if lc == 0:
                        ir = c_re[n][:, 0:1]
                        ii = c_im[n][:, 0:1]
                    else:
                        pwr, pwi = prev_w[np2]
                        ir = pwr[:, (jj + 1) * LC - 1:(jj + 1) * LC]
                        ii = pwi[:, (jj + 1) * LC - 1:(jj + 1) * LC]
                    nc.vector.tensor_tensor_scan(
                        wr[:, ts(jj, LC)], rbc, wr[:, ts(jj, LC)],
                        ir, MULT, ADD)
                    nc.vector.tensor_tensor_scan(
                        wi[:, ts(jj, LC)], rbc, wi[:, ts(jj, LC)],
                        ii, MULT, ADD)
                # demod: x = E * z, straight into matmul-ready bf16 tiles
                t5 = pm.tile([P, 2 * LC], BF16, tag="t1", name="t1")
                t6 = pm.tile([P, 2 * LC], BF16, tag="t2", name="t2")
                xrb = px.tile([P, 2 * LC], BF16, tag=f"xr{np2}",
                              name=f"xr{np2}")
                xib = px.tile([P, 2 * LC], BF16, tag=f"xi{np2}",
                              name=f"xi{np2}")
                nc.vector.tensor_mul(t5[:], cosr, wr[:])
                nc.vector.tensor_mul(t6[:], sinr, wi[:])
                nc.vector.tensor_sub(xrb[:], t5[:], t6[:])
                t7 = pm.tile([P, 2 * LC], BF16, tag="t1", name="t1")
                t8 = pm.tile([P, 2 * LC], BF16, tag="t2", name="t2")
                nc.vector.tensor_mul(t7[:], cosr, wi[:])
                nc.vector.tensor_mul(t8[:], sinr, wr[:])
                nc.vector.tensor_add(xib[:], t7[:], t8[:])
                xs[0].append(xrb)
                xs[1].append(xib)
                prev_w[np2] = (wr, wi)
            emit_y(uc, uf8s[lc], xs, lc)
    _split_waits(nc)
    return nc


def _prep(u, nu_log, theta_log, gamma_log, B_re, B_im, C_re, C_im, D):
    r = np.exp(-np.exp(nu_log.astype(np.float64)))
    theta = np.exp(theta_log.astype(np.float64))
    gamma = np.exp(gamma_log.astype(np.float64))
    t = np.arange(L, dtype=np.float64)
    ang = theta[:, None] * t[None, :]
    tab = np.empty((2, N, L), BF)
    tab[0] = np.cos(ang).astype(BF)
    tab[1] = np.sin(ang).astype(BF)
    rcol = r.astype(np.float32)[:, None].copy()
    bgt = np.empty((2, H, N), BF)
    bgt[0] = (gamma[:, None] * B_re).T.astype(BF)
    bgt[1] = (gamma[:, None] * B_im).T.astype(BF)
    ctt = np.empty((2, N, H), BF)
    ctt[0] = C_re.T.astype(BF)
    ctt[1] = (-C_im.T).astype(BF)
    dtn = np.ascontiguousarray(D.T)
    dtw = dtn.astype(BF)
    df8 = (dtn[FPK * 128:, :] * SF8).astype(E4)
    common = dict(bg=bgt, ct=ctt, dtw=dtw, df8=df8, tab=tab, rcol=rcol)
    in_maps = []
    for b in range(B):
        m = dict(common)
        ut = np.ascontiguousarray(u[b].T)
        m["ut"] = ut.astype(BF)
        m["uf8"] = (ut[FPK * 128:, :] / SF8).astype(E4)
        in_maps.append(m)
    return in_maps


def kernel(u, nu_log, theta_log, gamma_log, B_re, B_im, C_re, C_im, D,
           _trace=False):
    u, nu_log, theta_log, gamma_log, B_re, B_im, C_re, C_im, D = (
        np.asarray(a) for a in
        (u, nu_log, theta_log, gamma_log, B_re, B_im, C_re, C_im, D))
    if "nc" not in _COMPILED:
        _COMPILED["nc"] = _build()
    nc = _COMPILED["nc"]
    in_maps = _prep(u, nu_log, theta_log, gamma_log, B_re, B_im, C_re, C_im, D)
    res = None
    err = None
    for _attempt in range(4):
        try:
            res = run_bass_kernel_spmd(nc, in_maps, list(range(B)),
                                       trace=_trace)
            break
        except ModuleNotFoundError:
            # axon NTFF hook unavailable; force tracing off (BASS_TRACE
            # in the env would otherwise re-enable it) and rerun
            _trace = False
            os.environ["BASS_NEVER_TRACE"] = "1"
        except Exception as e:  # transient NRT / device hiccup: retry
            err = e
    if res is None:
        raise err if err is not None else RuntimeError("bass run failed")
    y = np.stack([res.results[i]["y"].astype(np.float32) for i in range(B)])
    kernel.last_exec_time_ns = res.exec_time_ns
    return y
